# revision 18
# baseline (speedup 1.0000x reference)
"""AttentiveReadout (gated segment-sum) Trainium2 kernel, v7.

pooled[b] = sum_{i: batch_id[i]==b} sigmoid(x[i] @ gate_w + gate_b) * x[i]

Strategy (8 NeuronCores, SPMD, memory-bound target):
  - batch_id sorted -> contiguous row ranges per 32-segment group.
    2048 segments = 64 groups of 32. Groups are SORTED BY SIZE and
    rank r goes to core r%8, slot r//8: every core's slot s has a
    similarly-sized group, so slot s's padded chunk count G_s is set
    by its own bucket max, not the global max (padding 3.2% -> 1.0%).
    Disjoint outputs, no collective; host unshuffles at gather.
  - Host folds the gate weight into x: x' = x * w, cast bf16 (halves
    HBM traffic; the kernel divides the pooled result by w at the
    end). logit_i = sum_d x'[i,d] becomes a plain row-sum.
  - Fine-grained HALF-GROUP pipeline (~18 ticks/core of <=32 chunks).
    Per tick: x DMA (2MB) || DVE fold-tree row-sums || ACT tail
    row-sums + sigmoid || GPSIMD local_scatter builds the sigma-scaled
    one-hot || prev tick's matmuls. Short per-tick chains keep the DMA
    stream saturated (the roofline). The last slot (smallest group)
    runs quarter-size ticks to shorten the drain tail.
  - One-hot lhsT via GPSIMD local_scatter: host precomputes int16
    indices idx[p,c] = c_local*32 + rel (negative = padding, ignored);
    the scatter zero-fills [P, h, 32] and writes sigma at the one-hot
    positions in ONE Pool op -- no iota/is_equal/mult, nothing on DVE.
  - TensorE matmul oht[:,c,:].T @ x' accumulates each slot's
    (32 segs, 256) PSUM slice (tile_position col = slot%4 avoids
    weight-tile thrash).
"""

import sys

if "/opt/trn_rl_repo" not in sys.path:
    sys.path.insert(0, "/opt/trn_rl_repo")

import numpy as np

N, D, B = 500000, 256, 2048
NCORES = 8
SEGS_PER_GROUP = 32
SEGS_PER_BLOCK = 128
GROUPS_PER_BLOCK = SEGS_PER_BLOCK // SEGS_PER_GROUP   # 4
NBLOCKS = B // SEGS_PER_BLOCK                         # 16
BLOCKS_PER_CORE = NBLOCKS // NCORES                   # 2
GROUPS_PER_CORE = BLOCKS_PER_CORE * GROUPS_PER_BLOCK  # 8
NGROUPS = B // SEGS_PER_GROUP                         # 64
P = 128
ACT_CHUNKS = 4     # chunks per tick whose row-sum runs on ScalarE


def _ticks(slot_Gs):
    """Per-core tick list [(s, lo, hi)]; last slot quarter-size."""
    ticks = []
    for s, Gs in enumerate(slot_Gs):
        if s == len(slot_Gs) - 1:
            q = (Gs + 3) // 4
            cuts = [0, q, 2 * q, 3 * q, Gs]
            for i in range(4):
                if cuts[i] < cuts[i + 1]:
                    ticks.append((s, cuts[i], cuts[i + 1]))
        else:
            h0 = (Gs + 1) // 2
            ticks.append((s, 0, h0))
            ticks.append((s, h0, Gs))
    return ticks


def _build_program(slot_Gs, gate_b_f, repeat=1, variant="full",
                   act_chunks=ACT_CHUNKS, dbg=(), xp_bufs=10, fp_bufs=1,
                   act_scrap="psum"):
    """variant: 'full' | 'nodma' (load x once, reuse for all ticks) |
    'dmaonly' (stream DMAs, one small consumer op per tick).
    dbg: subset of {'no_mm','no_fold','no_oh'} to skip components."""
    import concourse.bacc as bacc
    import concourse.mybir as mybir
    import concourse.tile as tile

    fp32 = mybir.dt.float32
    fp16 = mybir.dt.float16
    bf16 = mybir.dt.bfloat16
    i16 = mybir.dt.int16
    Alu = mybir.AluOpType
    GPB = GROUPS_PER_BLOCK
    SPG = SEGS_PER_GROUP
    HMAX = max((Gs + 1) // 2 for Gs in slot_Gs)
    offs = np.concatenate([[0], np.cumsum([P * Gs for Gs in slot_Gs])])
    TOT_ROWS = int(offs[-1])

    base_ticks = _ticks(slot_Gs)
    NT = len(base_ticks)
    ticks = base_ticks * repeat
    A = min(act_chunks, max(0, min((Gs + 1) // 2 for Gs in slot_Gs) - 2))

    nc = bacc.Bacc("TRN2", target_bir_lowering=False, debug=False,
                   num_devices=NCORES)

    x_dram = nc.dram_tensor("x", [TOT_ROWS, D], bf16,
                            kind="ExternalInput").ap()
    idx_dram = nc.dram_tensor("idx", [P, NT, SPG], i16,
                              kind="ExternalInput").ap()
    rw_dram = nc.dram_tensor("rw", [P, D], fp32, kind="ExternalInput").ap()
    out_dram = nc.dram_tensor("out", [BLOCKS_PER_CORE, SEGS_PER_BLOCK, D],
                              fp32, kind="ExternalOutput").ap()

    with tile.TileContext(nc) as tc:
        with (
            tc.tile_pool(name="consts", bufs=1) as consts,
            tc.tile_pool(name="xp", bufs=xp_bufs) as xp,
            tc.tile_pool(name="logp", bufs=4) as logp,
            tc.tile_pool(name="stp", bufs=4) as stp,
            tc.tile_pool(name="fp", bufs=fp_bufs) as fp,
            tc.tile_pool(name="scrap", bufs=2,
                         space="PSUM" if act_scrap == "psum" else "SBUF"
                         ) as scrap,
            tc.tile_pool(name="ohp", bufs=3) as ohp,
            tc.tile_pool(name="outp", bufs=2) as outp,
            tc.tile_pool(name="psump", bufs=2, space="PSUM") as psump,
        ):
            rw_t = consts.tile([P, D], fp32)
            nc.scalar.dma_start(rw_t[:], rw_dram[:])
            bias_t = consts.tile([P, 1], fp32)
            nc.gpsimd.memset(bias_t[:], gate_b_f)
            scr = consts.tile([P, 8], bf16)
            idx_all = consts.tile([P, NT, SPG], i16)
            nc.scalar.dma_start(idx_all[:], idx_dram[:])

            def fold(xt, logt, lo, hi):
                """Row-sum chunks [lo,hi) of this tick's xt (local idx)."""
                if "no_fold" in dbg:
                    return nc.vector.memset(logt[:, lo:hi], 0.5)
                h = hi - lo
                f1 = fp.tile([P, HMAX, 128], fp16, tag="f1")
                nc.vector.tensor_tensor(
                    out=f1[:, :h, :], in0=xt[:, lo:hi, 0:128],
                    in1=xt[:, lo:hi, 128:256], op=Alu.add)
                f2 = fp.tile([P, HMAX, 64], fp16, tag="f2")
                nc.vector.tensor_tensor(
                    out=f2[:, :h, :], in0=f1[:, :h, 0:64],
                    in1=f1[:, :h, 64:128], op=Alu.add)
                f3 = fp.tile([P, HMAX, 32], fp16, tag="f3")
                nc.vector.tensor_tensor(
                    out=f3[:, :h, :], in0=f2[:, :h, 0:32],
                    in1=f2[:, :h, 32:64], op=Alu.add)
                f4 = fp.tile([P, HMAX, 16], fp16, tag="f4")
                nc.vector.tensor_tensor(
                    out=f4[:, :h, :], in0=f3[:, :h, 0:16],
                    in1=f3[:, :h, 16:32], op=Alu.add)
                f5 = fp.tile([P, HMAX, 8], fp16, tag="f5")
                nc.vector.tensor_tensor(
                    out=f5[:, :h, :], in0=f4[:, :h, 0:8],
                    in1=f4[:, :h, 8:16], op=Alu.add)
                nc.vector.tensor_reduce(
                    out=logt[:, lo:hi], in_=f5[:, :h, :],
                    axis=mybir.AxisListType.X, op=Alu.add)

            def matmuls(prev_state):
                """Prev tick's matmuls (lhsT rows of its scattered
                one-hot)."""
                pxt, poht, ppsum, ps, plo, phi = prev_state
                if "no_mm" in dbg:
                    return
                base = (ps % GPB) * SPG
                Gs = slot_Gs[ps]
                for c in range(phi - plo):
                    gc = plo + c
                    nc.tensor.matmul(
                        ppsum[base:base + SPG, :], poht[:, c, :],
                        pxt[:, c, :],
                        start=(gc == 0), stop=(gc == Gs - 1),
                        tile_position=(0, base))

            def flush_block(psum_t, blk):
                if "no_mm" in dbg:
                    return nc.scalar.dma_start(out_dram[blk], rw_t[:])
                out_t = outp.tile([SEGS_PER_BLOCK, D], fp32, tag="out_t")
                nc.vector.tensor_tensor(
                    out=out_t[:], in0=psum_t[:], in1=rw_t[:], op=Alu.mult)
                nc.scalar.dma_start(out_dram[blk], out_t[:])

            xt_res = None
            if variant == "nodma":
                xt_res = consts.tile([P, HMAX, D], bf16)
                nc.sync.dma_start(
                    xt_res[:],
                    x_dram[0:P * HMAX, :].rearrange("(p c) d -> p c d", p=P))

            prev = None
            psum_t = None
            for tick_i, (s, lo, hi) in enumerate(ticks):
                h = hi - lo
                Gs = slot_Gs[s]
                ti = tick_i % NT
                blk = s // GPB
                if s % GPB == 0 and lo == 0 and variant != "dmaonly":
                    psum_t = psump.tile([SEGS_PER_BLOCK, D], fp32,
                                        tag="psum_t")
                if variant == "nodma":
                    xt = xt_res
                else:
                    xt = xp.tile([P, HMAX, D], bf16, tag="xt")
                    nc.sync.dma_start(
                        xt[:, :h, :],
                        x_dram[int(offs[s]):int(offs[s + 1]), :]
                        .rearrange("(p c) d -> p c d", p=P)[:, lo:hi, :])
                logt = logp.tile([P, HMAX], fp32, tag="logt")
                if variant == "dmaonly":
                    nc.vector.tensor_scalar(
                        out=scr[:, 0:4], in0=xt[:, 0, 0:4],
                        scalar1=1.0, scalar2=None, op0=Alu.mult,
                        op1=Alu.add, accum_out=logt[:, 4:5])
                    if tick_i == len(ticks) - 1:
                        nc.scalar.dma_start(out_dram[blk], rw_t[:])
                    continue

                # prev tick's matmuls run while this tick's row-sums
                # proceed on DVE/ACT
                if prev is not None:
                    matmuls(prev)
                    if prev[3] % GPB == GPB - 1 and \
                            prev[5] == slot_Gs[prev[3]]:
                        flush_block(prev[2], prev[3] // GPB)

                # ScalarE tail chunks (Copy + accum into logt)
                sdt = fp32 if act_scrap == "psum" else bf16
                for c in range(h - A, h):
                    scr_a = scrap.tile([P, D], sdt, tag="scra")
                    nc.scalar.activation(
                        scr_a[:], xt[:, c, :],
                        mybir.ActivationFunctionType.Copy,
                        accum_out=logt[:, c:c + 1])
                fold(xt, logt, 0, h - A)

                st = stp.tile([P, SPG], bf16, tag="st")
                nc.scalar.activation(
                    st[:, :h], logt[:, :h],
                    mybir.ActivationFunctionType.Sigmoid, bias=bias_t[:])

                # sigma-scaled one-hot in ONE GPSIMD op: zero-fill +
                # scatter st at idx (padding rows have negative idx)
                oht = ohp.tile([P, HMAX, SPG], bf16, tag="oht")
                if "no_oh" in dbg:
                    nc.vector.memset(oht[:, :h, :], 0.0)
                else:
                    nc.gpsimd.local_scatter(
                        oht[:, :h, :], st[:], idx_all[:, ti, :],
                        channels=P, num_elems=h * SPG, num_idxs=SPG)
                prev = (xt, oht, psum_t, s, lo, hi)

            if variant != "dmaonly":
                matmuls(prev)
                flush_block(prev[2], prev[3] // GPB)

    nc.compile()
    return nc


def _prep_inputs(x, batch_id, gate_w):
    """Shard + pad + fold w on host. Returns (in_maps, slot_Gs, order)."""
    import ml_dtypes

    bid = np.asarray(batch_id).astype(np.int64)
    x = np.asarray(x, dtype=np.float32)
    w = np.asarray(gate_w, np.float32).reshape(D)
    bounds = np.searchsorted(bid, np.arange(NGROUPS + 1) * SEGS_PER_GROUP)
    sizes = bounds[1:] - bounds[:-1]
    order = np.argsort(-sizes, kind="stable")
    slot_Gs = [max(2, -(-int(sizes[order[s * NCORES:(s + 1) * NCORES]]
                            .max()) // P))
               for s in range(GROUPS_PER_CORE)]
    offs = np.concatenate([[0], np.cumsum([P * Gs for Gs in slot_Gs])])
    TOT_ROWS = int(offs[-1])

    xw = (x * w[None, :]).astype(ml_dtypes.bfloat16)
    rw = np.broadcast_to((1.0 / w).astype(np.float32).reshape(1, D),
                         (P, D)).copy()
    base_ticks = _ticks(slot_Gs)
    NT = len(base_ticks)
    kk_ar = np.arange(SEGS_PER_GROUP)

    in_maps = []
    for k in range(NCORES):
        x_pad = np.zeros((TOT_ROWS, D), ml_dtypes.bfloat16)
        idx = np.empty((P, NT, SEGS_PER_GROUP), np.int16)
        rel_slot = {}
        for s in range(GROUPS_PER_CORE):
            gg = int(order[s * NCORES + k])
            lo, hi = int(bounds[gg]), int(bounds[gg + 1])
            nrow = hi - lo
            x_pad[int(offs[s]):int(offs[s]) + nrow] = xw[lo:hi]
            rel = np.full(P * slot_Gs[s], -1, np.int32)
            rel[:nrow] = (bid[lo:hi] - gg * SEGS_PER_GROUP).astype(np.int32)
            rel_slot[s] = rel.reshape(P, slot_Gs[s])
        for t, (s, tlo, thi) in enumerate(base_ticks):
            c_glob = tlo + kk_ar
            valid_c = c_glob < thi
            rel = rel_slot[s][:, np.minimum(c_glob, slot_Gs[s] - 1)]
            val = valid_c[None, :] & (rel >= 0)
            idx[:, t, :] = np.where(
                val, kk_ar[None, :] * SEGS_PER_GROUP + rel,
                -1 - kk_ar[None, :])
        in_maps.append({"x": x_pad, "idx": idx, "rw": rw})
    return in_maps, slot_Gs, order


def kernel(x, batch_id, batch_size, gate_w, gate_b, _ret_extra=False):
    from concourse.bass_utils import run_bass_kernel_spmd

    gate_b_f = float(np.asarray(gate_b).reshape(-1)[0])
    in_maps, slot_Gs, order = _prep_inputs(x, batch_id, gate_w)
    nc = _build_program(slot_Gs, gate_b_f)
    core_ids = list(range(NCORES))
    res = run_bass_kernel_spmd(nc, in_maps, core_ids)
    out = np.empty((B, D), np.float32)
    for k in core_ids:
        rows = res.results[k]["out"].reshape(
            BLOCKS_PER_CORE * SEGS_PER_BLOCK, D)
        for s in range(GROUPS_PER_CORE):
            gg = int(order[s * NCORES + k])
            out[gg * SEGS_PER_GROUP:(gg + 1) * SEGS_PER_GROUP] = \
                rows[s * SEGS_PER_GROUP:(s + 1) * SEGS_PER_GROUP]
    if _ret_extra:
        return out, (nc, in_maps)
    return out


if __name__ == "__main__":
    rng = np.random.default_rng(0)
    x = rng.standard_normal((N, D), dtype=np.float32)
    bid = np.sort(rng.integers(0, B, N)).astype(np.int64)
    gw = (rng.standard_normal((D, 1), dtype=np.float32) / 16.0)
    gb = np.zeros((1,), np.float32)
    out = kernel(x, bid, B, gw, gb)
    w = np.asarray(gw, np.float64).reshape(D)
    s = 1.0 / (1.0 + np.exp(-(x.astype(np.float64) @ w + float(gb[0]))))
    weighted = x.astype(np.float64) * s[:, None]
    ref = np.zeros((B, D), np.float64)
    np.add.at(ref, bid, weighted)
    err = np.abs(out - ref).max() / np.abs(ref).max()
    rel = np.linalg.norm(out - ref) / np.linalg.norm(ref)
    print("abs-rel max err:", err, " fro rel err:", rel)
